# revision 7
# baseline (speedup 1.0000x reference)
"""Trainium2 Bass kernel for CMELossAngularProfileMSE_V2.

Strategy (pure data parallel over batch, 8 NeuronCores):
  - Shard B=128 samples -> 16 per core.
  - Per core, per sample: DMA the full [128, 5760] tile with a single
    dma_start (r-major within partition: partition p holds r in
    [16p, 16p+16), free dim = 16*360 contiguous; 23KB/partition
    descriptors halve DGE descriptor load vs half-sample chunks).
  - Fold 16 q-slices with a single DVE tree of just 3 ops
    (2880+1440+360-wide adds, ~5.3us/sample) leaving THREE slices;
    the TensorE absorbs the remaining merges: 3 one-hot matmuls
    accumulate the slices into PSUM row b (~4.8us/sample, one
    accumulation group). Every engine sits under the ~6.3us/sample
    wire cadence, so the pipeline is robustly DMA-bound (the old
    all-DVE fold at ~6.4us/sample was the limiter and its 5% deficit
    made cores collapse into stall limit-cycles; a GPSIMD offload
    fails differently: concurrent GPSIMD slows DVE ~4x via SBUF
    contention).
  - Host precomputes T' = R*T and w' = w/R^2 (exact power-of-two
    scalings of the Gaussian target / distance weight derived from
    theta_min/theta_max), so the device epilogue is just
    sum_theta((S - T')^2 * w') per sample -> out [16, 1], on DVE.
  - Host: loss = sum(all per-sample sums) / (360 * 128).
"""
import numpy as np

import concourse.bacc as bacc
import concourse.tile as tile
from concourse import mybir
from concourse.bass_utils import run_bass_kernel_spmd

F32 = mybir.dt.float32
ADD = mybir.AluOpType.add

N_CORES = 8
B = 128            # full batch
BS = B // N_CORES  # samples per core (16)
R = 2048
TH = 360
Q = 16             # r-slices per partition (2048 = 128 * 16)
SIGMA = 10.0
ALPHA_WEIGHT = 2.0
LAMBDA_ANG = 1.0

H = (Q // 2) * TH  # half-sample width (2880)


def _build_nc():
    nc = bacc.Bacc("TRN2", target_bir_lowering=False, debug=False)
    x = nc.dram_tensor("x", [BS, 128, Q * TH], F32, kind="ExternalInput").ap()
    tw = nc.dram_tensor("tw", [2, BS, TH], F32, kind="ExternalInput").ap()
    out = nc.dram_tensor("out", [BS, 1], F32, kind="ExternalOutput").ap()

    from contextlib import ExitStack
    with tile.TileContext(nc) as tc, ExitStack() as ctx:
        consts = ctx.enter_context(tc.tile_pool(name="consts", bufs=1))
        inp = ctx.enter_context(tc.tile_pool(name="inp", bufs=8))
        psum = ctx.enter_context(tc.tile_pool(name="psum", bufs=1, space="PSUM"))
        small = ctx.enter_context(tc.tile_pool(name="small", bufs=1))

        # one-hot weight matrices: O[:, b, j] = 1 if j == b else 0
        # (PSUM out base partition must be 0/32/64, so per-row matmuls
        # need the one-hot trick; built on gpsimd, idle at startup)
        O = consts.tile([128, BS, BS], F32)
        nc.gpsimd.memset(O[:], 0.0)
        for b in range(BS):
            nc.gpsimd.memset(O[:, b, b:b + 1], 1.0)

        t16w16 = small.tile([BS, 2, TH], F32)
        t16 = t16w16[:, 0, :]
        w16 = t16w16[:, 1, :]

        ps = psum.tile([BS, TH], F32)
        for b in range(BS):
            xt = inp.tile([128, Q * TH], F32)
            if b < BS - 1:
                # steady state: one full-sample DMA; the fold tree needs
                # the whole sample anyway
                nc.sync.dma_start(xt[:], x[b])
                # 3-op DVE tree -> 3 surviving slices
                nc.vector.tensor_add(xt[:, 0:2880], xt[:, 0:2880],
                                     xt[:, 2880:5760])
                nc.vector.tensor_add(xt[:, 0:1440], xt[:, 0:1440],
                                     xt[:, 1440:2880])
                nc.vector.tensor_add(xt[:, 0:360], xt[:, 0:360],
                                     xt[:, 360:720])
                # slices at [0:360], [720:1080], [1080:1440] -> row b
                nc.tensor.matmul(ps[:], O[:, b, :], xt[:, 0:360],
                                 start=(b == 0), stop=False)
                nc.tensor.matmul(ps[:], O[:, b, :], xt[:, 720:1080],
                                 start=False, stop=False)
                nc.tensor.matmul(ps[:], O[:, b, :], xt[:, 1080:1440],
                                 start=False, stop=False)
                if b == 0:
                    # tw load dispatched early (second in queue) so it is
                    # resident long before the epilogue; at the queue tail
                    # it stalled the epilogue ~17us
                    nc.sync.dma_start(t16w16[:],
                                      tw.rearrange("two b t -> b two t"))
            else:
                # last sample: diminishing chunks + eager partial folds so
                # only ~1us of fold work trails the final byte
                nc.sync.dma_start(xt[:, 0:H], x[b][:, 0:H])
                nc.sync.dma_start(xt[:, 2880:4320], x[b][:, 2880:4320])
                nc.sync.dma_start(xt[:, 4320:5040], x[b][:, 4320:5040])
                nc.sync.dma_start(xt[:, 5040:5760], x[b][:, 5040:5760])
                # H0 folds to 3 slices as soon as it lands; matmul early
                nc.vector.tensor_add(xt[:, 0:1440], xt[:, 0:1440],
                                     xt[:, 1440:2880])
                nc.vector.tensor_add(xt[:, 0:360], xt[:, 0:360],
                                     xt[:, 360:720])
                nc.tensor.matmul(ps[:], O[:, b, :], xt[:, 0:360],
                                 start=False, stop=False)
                nc.tensor.matmul(ps[:], O[:, b, :], xt[:, 720:1080],
                                 start=False, stop=False)
                nc.tensor.matmul(ps[:], O[:, b, :], xt[:, 1080:1440],
                                 start=False, stop=False)
                # Q2 (slices 8-11) -> s at [2880:3240]
                nc.vector.tensor_add(xt[:, 2880:3600], xt[:, 2880:3600],
                                     xt[:, 3600:4320])
                nc.vector.tensor_add(xt[:, 2880:3240], xt[:, 2880:3240],
                                     xt[:, 3240:3600])
                # E6 (slices 12,13)
                nc.vector.tensor_add(xt[:, 4320:4680], xt[:, 4320:4680],
                                     xt[:, 4680:5040])
                nc.vector.tensor_add(xt[:, 2880:3240], xt[:, 2880:3240],
                                     xt[:, 4320:4680])
                # E7 (slices 14,15) is the last chunk on the wire
                nc.vector.tensor_add(xt[:, 5040:5400], xt[:, 5040:5400],
                                     xt[:, 5400:5760])
                nc.vector.tensor_add(xt[:, 2880:3240], xt[:, 2880:3240],
                                     xt[:, 5040:5400])
                nc.tensor.matmul(ps[:], O[:, b, :], xt[:, 2880:3240],
                                 start=False, stop=True)

        d16 = small.tile([BS, TH], F32)
        nc.vector.scalar_tensor_tensor(
            d16[:], ps[:], 1.0, t16,
            op0=mybir.AluOpType.mult, op1=mybir.AluOpType.subtract,
        )
        sq16 = small.tile([BS, TH], F32)
        nc.vector.scalar_tensor_tensor(
            sq16[:], d16[:], 1.0, d16[:],
            op0=mybir.AluOpType.mult, op1=mybir.AluOpType.mult,
        )
        sqw16 = small.tile([BS, TH], F32)
        red = small.tile([BS, 1], F32)
        nc.vector.scalar_tensor_tensor(
            sqw16[:], sq16[:], 1.0, w16,
            op0=mybir.AluOpType.mult, op1=mybir.AluOpType.mult,
            accum_out=red[:],
        )
        nc.sync.dma_start(out[:], red[:])
    nc.compile()
    return nc


def _target_and_weight(theta_min: np.ndarray, theta_max: np.ndarray):
    """Gaussian soft target T and distance weight w, [B, TH] float32 each.

    Mirrors the reference formulas (computed in float64, cast to float32;
    differences vs the f32 jax pipeline are O(1 ulp))."""
    theta = np.arange(TH, dtype=np.float64)[None, None, :]      # [1, 1, TH]
    tmin = theta_min.astype(np.float64)[:, :, None]             # [B, K, 1]
    tmax = theta_max.astype(np.float64)[:, :, None]

    center_wrap = np.mod(0.5 * (tmin + tmax + 360.0), 360.0)
    center_t = np.where(tmin <= tmax, 0.5 * (tmin + tmax), center_wrap)
    d = np.abs(theta - center_t)
    dist_t = np.minimum(d, 360.0 - d)                           # [B, K, TH]
    T = np.clip(np.exp(-0.5 * (dist_t / SIGMA) ** 2).sum(axis=1), 0.0, 1.0)

    center_w = (tmin + np.mod(tmax - tmin, 360.0)) / 2.0
    dw = np.abs(theta - center_w)
    dist_w = np.minimum(dw, 360.0 - dw)
    w = 1.0 + ALPHA_WEIGHT * (dist_w.max(axis=1) / 180.0)       # [B, TH]

    # Feed the device T' = R*T and w' = w/R^2 (both exact scalings by
    # powers of two) so it can use the raw radial sums S instead of the
    # mean A = S/R:  ((S - R*T)^2 * w/R^2) == ((A - T)^2 * w).
    Tp = (T * np.float32(R)).astype(np.float32)
    wp = (w / np.float32(R) ** 2).astype(np.float32)
    return Tp, wp


_NC_CACHE = None


def _get_nc():
    global _NC_CACHE
    if _NC_CACHE is None:
        _NC_CACHE = _build_nc()
    return _NC_CACHE


def _run(mask_pred, theta_min, theta_max, trace=False, trace_kwargs=None,
         trace_cores=None):
    mask_pred = np.asarray(mask_pred, dtype=np.float32)
    theta_min = np.asarray(theta_min)
    theta_max = np.asarray(theta_max)
    T, w = _target_and_weight(theta_min, theta_max)

    in_maps = []
    for i in range(N_CORES):
        sl = slice(i * BS, (i + 1) * BS)
        x_core = np.ascontiguousarray(mask_pred[sl, 0]).reshape(BS, 128, Q * TH)
        tw_core = np.stack([T[sl], w[sl]])
        in_maps.append({"x": x_core, "tw": tw_core})

    kwargs = {}
    if trace:
        kwargs["trace"] = True
        if trace_kwargs:
            kwargs["trace_kwargs"] = trace_kwargs
        if trace_cores is not None:
            kwargs["trace_cores"] = trace_cores
    res = run_bass_kernel_spmd(_get_nc(), in_maps, core_ids=list(range(N_CORES)),
                               **kwargs)
    per_sample = np.concatenate(
        [res.results[i]["out"][:, 0] for i in range(N_CORES)]
    )
    total = per_sample.astype(np.float64).sum() / (TH * B)
    return np.float32(LAMBDA_ANG * total), res


def kernel(mask_pred: np.ndarray, theta_min: np.ndarray,
           theta_max: np.ndarray) -> np.ndarray:
    loss, _ = _run(mask_pred, theta_min, theta_max)
    return np.asarray(loss, dtype=np.float32)


# revision 10
# speedup vs baseline: 1.0699x; 1.0699x over previous
"""Trainium2 Bass kernel for CMELossAngularProfileMSE_V2.

Strategy (pure data parallel over batch, 8 NeuronCores):
  - Shard B=128 samples -> 16 per core.
  - Per core, per sample: DMA the full [128, 5760] tile with a single
    dma_start (r-major within partition: partition p holds r in
    [16p, 16p+16), free dim = 16*360 contiguous; 23KB/partition
    descriptors halve DGE descriptor load vs half-sample chunks).
  - Fold 16 q-slices with a single DVE tree of just 3 ops
    (2880+1440+360-wide adds, ~5.3us/sample) leaving THREE slices;
    the TensorE absorbs the remaining merges: 3 one-hot matmuls
    accumulate the slices into PSUM row b (~4.8us/sample, one
    accumulation group). Every engine sits under the ~6.3us/sample
    wire cadence, so the pipeline is robustly DMA-bound (the old
    all-DVE fold at ~6.4us/sample was the limiter and its 5% deficit
    made cores collapse into stall limit-cycles; a GPSIMD offload
    fails differently: concurrent GPSIMD slows DVE ~4x via SBUF
    contention).
  - Host precomputes T' = R*T and w' = w/R^2 (exact power-of-two
    scalings of the Gaussian target / distance weight derived from
    theta_min/theta_max), so the device epilogue is just
    sum_theta((S - T')^2 * w') per sample -> out [16, 1], on DVE.
  - Host: loss = sum(all per-sample sums) / (360 * 128).
"""
import numpy as np

import concourse.bacc as bacc
import concourse.tile as tile
from concourse import mybir
from concourse.bass_utils import run_bass_kernel_spmd

F32 = mybir.dt.float32
ADD = mybir.AluOpType.add

N_CORES = 8
B = 128            # full batch
BS = B // N_CORES  # samples per core (16)
R = 2048
TH = 360
Q = 16             # r-slices per partition (2048 = 128 * 16)
SIGMA = 10.0
ALPHA_WEIGHT = 2.0
LAMBDA_ANG = 1.0

H = (Q // 2) * TH  # half-sample width (2880)


def _build_nc():
    nc = bacc.Bacc("TRN2", target_bir_lowering=False, debug=False)
    x = nc.dram_tensor("x", [BS, 128, Q * TH], F32, kind="ExternalInput").ap()
    tw = nc.dram_tensor("tw", [2, BS, TH], F32, kind="ExternalInput").ap()
    out = nc.dram_tensor("out", [BS, 1], F32, kind="ExternalOutput").ap()

    from contextlib import ExitStack
    with tile.TileContext(nc) as tc, ExitStack() as ctx:
        consts = ctx.enter_context(tc.tile_pool(name="consts", bufs=1))
        inp = ctx.enter_context(tc.tile_pool(name="inp", bufs=6))
        tailp = ctx.enter_context(tc.tile_pool(name="tailp", bufs=1))
        psum = ctx.enter_context(tc.tile_pool(name="psum", bufs=1, space="PSUM"))
        small = ctx.enter_context(tc.tile_pool(name="small", bufs=1))

        # one-hot weight matrices: O[:, b, j] = 1 if j == b else 0
        # (PSUM out base partition must be 0/32/64, so per-row matmuls
        # need the one-hot trick; built on gpsimd, idle at startup)
        O = consts.tile([128, BS, BS], F32)
        nc.gpsimd.memset(O[:], 0.0)
        for b in range(BS):
            nc.gpsimd.memset(O[:, b, b:b + 1], 1.0)

        t16w16 = small.tile([BS, 2, TH], F32)
        t16 = t16w16[:, 0, :]
        w16 = t16w16[:, 1, :]

        ps = psum.tile([BS, TH], F32)

        def tree3(xt, b, start):
            """3-op DVE tree -> 3 slices -> 3 accumulating matmuls."""
            nc.vector.tensor_add(xt[:, 0:2880], xt[:, 0:2880],
                                 xt[:, 2880:5760])
            nc.vector.tensor_add(xt[:, 0:1440], xt[:, 0:1440],
                                 xt[:, 1440:2880])
            nc.vector.tensor_add(xt[:, 0:360], xt[:, 0:360],
                                 xt[:, 360:720])
            nc.tensor.matmul(ps[:], O[:, b, :], xt[:, 0:360],
                             start=start, stop=False)
            nc.tensor.matmul(ps[:], O[:, b, :], xt[:, 720:1080],
                             start=False, stop=False)
            nc.tensor.matmul(ps[:], O[:, b, :], xt[:, 1080:1440],
                             start=False, stop=False)

        # samples 0..13: one full-sample DMA each (the fold tree needs the
        # whole sample anyway; 23KB/partition descriptors halve DGE load)
        tiles = {}
        for b in range(BS - 2):
            xt = inp.tile([128, Q * TH], F32)
            nc.sync.dma_start(xt[:], x[b])
            if b == 0:
                # tw load dispatched early (second in queue) so it is
                # resident long before the epilogue needs it
                nc.sync.dma_start(t16w16[:],
                                  tw.rearrange("two b t -> b two t"))
            tree3(xt, b, start=(b == 0))

        # Last two samples: diminishing chunks with dispatch order ==
        # fold program order, so the in-order DVE queue drains folds as
        # chunks arrive and only ~1us of work trails the final byte.
        # (A single specialized last sample does not work: its "eager"
        # folds sit behind sample 14's folds, whose data arrives at the
        # very end, serializing an extra full fold+matmul into the tail.)
        xa = tailp.tile([128, Q * TH], F32)   # sample 14
        xb = tailp.tile([128, Q * TH], F32)   # sample 15
        ba, bb = BS - 2, BS - 1
        # interleaved chunk dispatch: H0(2880) Q2(1440) E6(720) E7(720)
        for xt, b in ((xa, ba), (xb, bb)):
            nc.sync.dma_start(xt[:, 0:H], x[b][:, 0:H])
        for xt, b in ((xa, ba), (xb, bb)):
            nc.sync.dma_start(xt[:, 2880:4320], x[b][:, 2880:4320])
        for xt, b in ((xa, ba), (xb, bb)):
            nc.sync.dma_start(xt[:, 4320:5040], x[b][:, 4320:5040])
        for xt, b in ((xa, ba), (xb, bb)):
            nc.sync.dma_start(xt[:, 5040:5760], x[b][:, 5040:5760])
        # H0 -> 3 slices (A' C D) -> 3 matmuls, as each H0 lands
        for xt, b in ((xa, ba), (xb, bb)):
            nc.vector.tensor_add(xt[:, 0:1440], xt[:, 0:1440],
                                 xt[:, 1440:2880])
            nc.vector.tensor_add(xt[:, 0:360], xt[:, 0:360],
                                 xt[:, 360:720])
            nc.tensor.matmul(ps[:], O[:, b, :], xt[:, 0:360],
                             start=False, stop=False)
            nc.tensor.matmul(ps[:], O[:, b, :], xt[:, 720:1080],
                             start=False, stop=False)
            nc.tensor.matmul(ps[:], O[:, b, :], xt[:, 1080:1440],
                             start=False, stop=False)
        # Q2 (slices 8-11) -> s1 at [2880:3240]
        for xt, b in ((xa, ba), (xb, bb)):
            nc.vector.tensor_add(xt[:, 2880:3600], xt[:, 2880:3600],
                                 xt[:, 3600:4320])
            nc.vector.tensor_add(xt[:, 2880:3240], xt[:, 2880:3240],
                                 xt[:, 3240:3600])
        # E6 (slices 12,13) folded into s1
        for xt, b in ((xa, ba), (xb, bb)):
            nc.vector.tensor_add(xt[:, 4320:4680], xt[:, 4320:4680],
                                 xt[:, 4680:5040])
            nc.vector.tensor_add(xt[:, 2880:3240], xt[:, 2880:3240],
                                 xt[:, 4320:4680])
        # E7 (slices 14,15): the last bytes on the wire
        for xt, b in ((xa, ba), (xb, bb)):
            nc.vector.tensor_add(xt[:, 5040:5400], xt[:, 5040:5400],
                                 xt[:, 5400:5760])
            nc.vector.tensor_add(xt[:, 2880:3240], xt[:, 2880:3240],
                                 xt[:, 5040:5400])
            nc.tensor.matmul(ps[:], O[:, b, :], xt[:, 2880:3240],
                             start=False, stop=(b == BS - 1))

        d16 = small.tile([BS, TH], F32)
        nc.vector.scalar_tensor_tensor(
            d16[:], ps[:], 1.0, t16,
            op0=mybir.AluOpType.mult, op1=mybir.AluOpType.subtract,
        )
        sq16 = small.tile([BS, TH], F32)
        nc.vector.scalar_tensor_tensor(
            sq16[:], d16[:], 1.0, d16[:],
            op0=mybir.AluOpType.mult, op1=mybir.AluOpType.mult,
        )
        sqw16 = small.tile([BS, TH], F32)
        red = small.tile([BS, 1], F32)
        nc.vector.scalar_tensor_tensor(
            sqw16[:], sq16[:], 1.0, w16,
            op0=mybir.AluOpType.mult, op1=mybir.AluOpType.mult,
            accum_out=red[:],
        )
        nc.sync.dma_start(out[:], red[:])
    nc.compile()
    return nc


def _target_and_weight(theta_min: np.ndarray, theta_max: np.ndarray):
    """Gaussian soft target T and distance weight w, [B, TH] float32 each.

    Mirrors the reference formulas (computed in float64, cast to float32;
    differences vs the f32 jax pipeline are O(1 ulp))."""
    theta = np.arange(TH, dtype=np.float64)[None, None, :]      # [1, 1, TH]
    tmin = theta_min.astype(np.float64)[:, :, None]             # [B, K, 1]
    tmax = theta_max.astype(np.float64)[:, :, None]

    center_wrap = np.mod(0.5 * (tmin + tmax + 360.0), 360.0)
    center_t = np.where(tmin <= tmax, 0.5 * (tmin + tmax), center_wrap)
    d = np.abs(theta - center_t)
    dist_t = np.minimum(d, 360.0 - d)                           # [B, K, TH]
    T = np.clip(np.exp(-0.5 * (dist_t / SIGMA) ** 2).sum(axis=1), 0.0, 1.0)

    center_w = (tmin + np.mod(tmax - tmin, 360.0)) / 2.0
    dw = np.abs(theta - center_w)
    dist_w = np.minimum(dw, 360.0 - dw)
    w = 1.0 + ALPHA_WEIGHT * (dist_w.max(axis=1) / 180.0)       # [B, TH]

    # Feed the device T' = R*T and w' = w/R^2 (both exact scalings by
    # powers of two) so it can use the raw radial sums S instead of the
    # mean A = S/R:  ((S - R*T)^2 * w/R^2) == ((A - T)^2 * w).
    Tp = (T * np.float32(R)).astype(np.float32)
    wp = (w / np.float32(R) ** 2).astype(np.float32)
    return Tp, wp


_NC_CACHE = None


def _get_nc():
    global _NC_CACHE
    if _NC_CACHE is None:
        _NC_CACHE = _build_nc()
    return _NC_CACHE


def _run(mask_pred, theta_min, theta_max, trace=False, trace_kwargs=None,
         trace_cores=None):
    mask_pred = np.asarray(mask_pred, dtype=np.float32)
    theta_min = np.asarray(theta_min)
    theta_max = np.asarray(theta_max)
    T, w = _target_and_weight(theta_min, theta_max)

    in_maps = []
    for i in range(N_CORES):
        sl = slice(i * BS, (i + 1) * BS)
        x_core = np.ascontiguousarray(mask_pred[sl, 0]).reshape(BS, 128, Q * TH)
        tw_core = np.stack([T[sl], w[sl]])
        in_maps.append({"x": x_core, "tw": tw_core})

    kwargs = {}
    if trace:
        kwargs["trace"] = True
        if trace_kwargs:
            kwargs["trace_kwargs"] = trace_kwargs
        if trace_cores is not None:
            kwargs["trace_cores"] = trace_cores
    res = run_bass_kernel_spmd(_get_nc(), in_maps, core_ids=list(range(N_CORES)),
                               **kwargs)
    per_sample = np.concatenate(
        [res.results[i]["out"][:, 0] for i in range(N_CORES)]
    )
    total = per_sample.astype(np.float64).sum() / (TH * B)
    return np.float32(LAMBDA_ANG * total), res


def kernel(mask_pred: np.ndarray, theta_min: np.ndarray,
           theta_max: np.ndarray) -> np.ndarray:
    loss, _ = _run(mask_pred, theta_min, theta_max)
    return np.asarray(loss, dtype=np.float32)


# revision 14
# speedup vs baseline: 1.1150x; 1.0422x over previous
"""Trainium2 Bass kernel for CMELossAngularProfileMSE_V2.

Strategy (pure data parallel over batch, 8 NeuronCores):
  - Shard B=128 samples -> 16 per core.
  - Per core, per sample: DMA the full [128, 5760] tile with a single
    dma_start (r-major within partition: partition p holds r in
    [16p, 16p+16), free dim = 16*360 contiguous; 23KB/partition
    descriptors halve DGE descriptor load vs half-sample chunks).
  - Fold 16 q-slices with a single DVE tree of just 3 ops
    (2880+1440+360-wide adds, ~5.3us/sample) leaving THREE slices;
    the TensorE absorbs the remaining merges: 3 one-hot matmuls
    accumulate the slices into PSUM row b (~4.8us/sample, one
    accumulation group). Every engine sits under the ~6.3us/sample
    wire cadence, so the pipeline is robustly DMA-bound (the old
    all-DVE fold at ~6.4us/sample was the limiter and its 5% deficit
    made cores collapse into stall limit-cycles; a GPSIMD offload
    fails differently: concurrent GPSIMD slows DVE ~4x via SBUF
    contention).
  - Host precomputes T' = R*T and w' = w/R^2 (exact power-of-two
    scalings of the Gaussian target / distance weight derived from
    theta_min/theta_max), so the device epilogue is just
    sum_theta((S - T')^2 * w') per sample -> out [16, 1], on DVE.
  - Host: loss = sum(all per-sample sums) / (360 * 128).
"""
import numpy as np

import concourse.bacc as bacc
import concourse.tile as tile
from concourse import mybir
from concourse.bass_utils import run_bass_kernel_spmd

F32 = mybir.dt.float32
ADD = mybir.AluOpType.add

N_CORES = 8
B = 128            # full batch
# Static asymmetric sharding: this host has degraded SDMA engines in the
# banks of (historically) the even cores (E15/core6, E32/core4, E96/core2
# across observed runs): one engine per afflicted bank runs ~25% slow and
# bounds that core's stream at ~8.7us/sample vs 6.85 healthy. Descriptor->
# engine assignment is round-robin (verified), so bytes cannot be steered
# off a slow engine within a core; instead the even cores get 14 samples
# and the odd cores 18 (4*18 + 4*14 = 128), run as two concurrently
# dispatched single-program meshes.
HEAVY_CORES = [1, 3, 5, 7]
LIGHT_CORES = [0, 2, 4, 6]
BS_H = 18
BS_L = 14
R = 2048
TH = 360
Q = 16             # r-slices per partition (2048 = 128 * 16)
SIGMA = 10.0
ALPHA_WEIGHT = 2.0
LAMBDA_ANG = 1.0

H = (Q // 2) * TH  # half-sample width (2880)


def _build_nc(BS):
    nc = bacc.Bacc("TRN2", target_bir_lowering=False, debug=False)
    x = nc.dram_tensor("x", [BS, 128, Q * TH], F32, kind="ExternalInput").ap()
    tw = nc.dram_tensor("tw", [2, BS, TH], F32, kind="ExternalInput").ap()
    out = nc.dram_tensor("out", [BS, 1], F32, kind="ExternalOutput").ap()

    from contextlib import ExitStack
    with tile.TileContext(nc) as tc, ExitStack() as ctx:
        consts = ctx.enter_context(tc.tile_pool(name="consts", bufs=1))
        inp = ctx.enter_context(tc.tile_pool(name="inp", bufs=6))
        tailp = ctx.enter_context(tc.tile_pool(name="tailp", bufs=1))
        psum = ctx.enter_context(tc.tile_pool(name="psum", bufs=1, space="PSUM"))
        small = ctx.enter_context(tc.tile_pool(name="small", bufs=1))

        # one-hot weight matrices: O[:, b, j] = 1 if j == b else 0
        # (PSUM out base partition must be 0/32/64, so per-row matmuls
        # need the one-hot trick; built on gpsimd, idle at startup)
        O = consts.tile([128, BS, BS], F32)
        nc.gpsimd.memset(O[:], 0.0)
        for b in range(BS):
            nc.gpsimd.memset(O[:, b, b:b + 1], 1.0)

        t16w16 = small.tile([BS, 2, TH], F32)
        t16 = t16w16[:, 0, :]
        w16 = t16w16[:, 1, :]

        ps = psum.tile([BS, TH], F32)

        def tree3(xt, b, start):
            """3-op DVE tree -> 3 slices -> 3 accumulating matmuls."""
            nc.vector.tensor_add(xt[:, 0:2880], xt[:, 0:2880],
                                 xt[:, 2880:5760])
            nc.vector.tensor_add(xt[:, 0:1440], xt[:, 0:1440],
                                 xt[:, 1440:2880])
            nc.vector.tensor_add(xt[:, 0:360], xt[:, 0:360],
                                 xt[:, 360:720])
            nc.tensor.matmul(ps[:], O[:, b, :], xt[:, 0:360],
                             start=start, stop=False)
            nc.tensor.matmul(ps[:], O[:, b, :], xt[:, 720:1080],
                             start=False, stop=False)
            nc.tensor.matmul(ps[:], O[:, b, :], xt[:, 1080:1440],
                             start=False, stop=False)

        # samples 0..13: one full-sample DMA each (the fold tree needs the
        # whole sample anyway; 23KB/partition descriptors halve DGE load)
        tiles = {}
        for b in range(BS - 2):
            xt = inp.tile([128, Q * TH], F32)
            nc.sync.dma_start(xt[:], x[b])
            if b == 0:
                # tw load dispatched early (second in queue) so it is
                # resident long before the epilogue needs it
                nc.sync.dma_start(t16w16[:],
                                  tw.rearrange("two b t -> b two t"))
            tree3(xt, b, start=(b == 0))

        # Last two samples: diminishing chunks with dispatch order ==
        # fold program order, so the in-order DVE queue drains folds as
        # chunks arrive and only ~1us of work trails the final byte.
        # (A single specialized last sample does not work: its "eager"
        # folds sit behind sample 14's folds, whose data arrives at the
        # very end, serializing an extra full fold+matmul into the tail.)
        xa = tailp.tile([128, Q * TH], F32)   # sample 14
        xb = tailp.tile([128, Q * TH], F32)   # sample 15
        ba, bb = BS - 2, BS - 1
        # interleaved chunk dispatch: H0(2880) Q2(1440) E6(720) E7(720)
        for xt, b in ((xa, ba), (xb, bb)):
            nc.sync.dma_start(xt[:, 0:H], x[b][:, 0:H])
        for xt, b in ((xa, ba), (xb, bb)):
            nc.sync.dma_start(xt[:, 2880:4320], x[b][:, 2880:4320])
        for xt, b in ((xa, ba), (xb, bb)):
            nc.sync.dma_start(xt[:, 4320:5040], x[b][:, 4320:5040])
        for xt, b in ((xa, ba), (xb, bb)):
            nc.sync.dma_start(xt[:, 5040:5760], x[b][:, 5040:5760])
        # H0 -> 3 slices (A' C D) -> 3 matmuls, as each H0 lands
        for xt, b in ((xa, ba), (xb, bb)):
            nc.vector.tensor_add(xt[:, 0:1440], xt[:, 0:1440],
                                 xt[:, 1440:2880])
            nc.vector.tensor_add(xt[:, 0:360], xt[:, 0:360],
                                 xt[:, 360:720])
            nc.tensor.matmul(ps[:], O[:, b, :], xt[:, 0:360],
                             start=False, stop=False)
            nc.tensor.matmul(ps[:], O[:, b, :], xt[:, 720:1080],
                             start=False, stop=False)
            nc.tensor.matmul(ps[:], O[:, b, :], xt[:, 1080:1440],
                             start=False, stop=False)
        # Q2 (slices 8-11) -> s1 at [2880:3240]
        for xt, b in ((xa, ba), (xb, bb)):
            nc.vector.tensor_add(xt[:, 2880:3600], xt[:, 2880:3600],
                                 xt[:, 3600:4320])
            nc.vector.tensor_add(xt[:, 2880:3240], xt[:, 2880:3240],
                                 xt[:, 3240:3600])
        # E6 (slices 12,13) folded into s1
        for xt, b in ((xa, ba), (xb, bb)):
            nc.vector.tensor_add(xt[:, 4320:4680], xt[:, 4320:4680],
                                 xt[:, 4680:5040])
            nc.vector.tensor_add(xt[:, 2880:3240], xt[:, 2880:3240],
                                 xt[:, 4320:4680])
        # E7 (slices 14,15): the last bytes on the wire
        for xt, b in ((xa, ba), (xb, bb)):
            nc.vector.tensor_add(xt[:, 5040:5400], xt[:, 5040:5400],
                                 xt[:, 5400:5760])
            nc.vector.tensor_add(xt[:, 2880:3240], xt[:, 2880:3240],
                                 xt[:, 5040:5400])
            nc.tensor.matmul(ps[:], O[:, b, :], xt[:, 2880:3240],
                             start=False, stop=(b == BS - 1))

        d16 = small.tile([BS, TH], F32)
        nc.vector.scalar_tensor_tensor(
            d16[:], ps[:], 1.0, t16,
            op0=mybir.AluOpType.mult, op1=mybir.AluOpType.subtract,
        )
        sq16 = small.tile([BS, TH], F32)
        nc.vector.scalar_tensor_tensor(
            sq16[:], d16[:], 1.0, d16[:],
            op0=mybir.AluOpType.mult, op1=mybir.AluOpType.mult,
        )
        sqw16 = small.tile([BS, TH], F32)
        red = small.tile([BS, 1], F32)
        nc.vector.scalar_tensor_tensor(
            sqw16[:], sq16[:], 1.0, w16,
            op0=mybir.AluOpType.mult, op1=mybir.AluOpType.mult,
            accum_out=red[:],
        )
        nc.sync.dma_start(out[:], red[:])
    nc.compile()
    return nc


def _target_and_weight(theta_min: np.ndarray, theta_max: np.ndarray):
    """Gaussian soft target T and distance weight w, [B, TH] float32 each.

    Mirrors the reference formulas (computed in float64, cast to float32;
    differences vs the f32 jax pipeline are O(1 ulp))."""
    theta = np.arange(TH, dtype=np.float64)[None, None, :]      # [1, 1, TH]
    tmin = theta_min.astype(np.float64)[:, :, None]             # [B, K, 1]
    tmax = theta_max.astype(np.float64)[:, :, None]

    center_wrap = np.mod(0.5 * (tmin + tmax + 360.0), 360.0)
    center_t = np.where(tmin <= tmax, 0.5 * (tmin + tmax), center_wrap)
    d = np.abs(theta - center_t)
    dist_t = np.minimum(d, 360.0 - d)                           # [B, K, TH]
    T = np.clip(np.exp(-0.5 * (dist_t / SIGMA) ** 2).sum(axis=1), 0.0, 1.0)

    center_w = (tmin + np.mod(tmax - tmin, 360.0)) / 2.0
    dw = np.abs(theta - center_w)
    dist_w = np.minimum(dw, 360.0 - dw)
    w = 1.0 + ALPHA_WEIGHT * (dist_w.max(axis=1) / 180.0)       # [B, TH]

    # Feed the device T' = R*T and w' = w/R^2 (both exact scalings by
    # powers of two) so it can use the raw radial sums S instead of the
    # mean A = S/R:  ((S - R*T)^2 * w/R^2) == ((A - T)^2 * w).
    Tp = (T * np.float32(R)).astype(np.float32)
    wp = (w / np.float32(R) ** 2).astype(np.float32)
    return Tp, wp


_NC_CACHE = {}


def _get_nc(bs):
    if bs not in _NC_CACHE:
        _NC_CACHE[bs] = _build_nc(bs)
    return _NC_CACHE[bs]


def _dispatch(nc, in_maps, dev_ids):
    """Adapted from bass2jax.run_bass_via_pjrt's multi-core path, but with
    an explicit device list so two different programs can run concurrently
    on disjoint device sets. Returns lazy jax arrays (async dispatch)."""
    import jax
    import jax.core
    from jax.experimental.shard_map import shard_map
    from jax.sharding import Mesh, PartitionSpec
    from concourse import mybir as _mybir
    from concourse.bass2jax import (
        _bass_exec_p, install_neuronx_cc_hook, partition_id_tensor)

    install_neuronx_cc_hook()
    partition_name = (nc.partition_id_tensor.name
                      if nc.partition_id_tensor else None)

    in_names, out_names, out_avals, zero_outs = [], [], [], []
    for alloc in nc.m.functions[0].allocations:
        if not isinstance(alloc, _mybir.MemoryLocationSet):
            continue
        name = alloc.memorylocations[0].name
        if alloc.kind == "ExternalInput":
            if name != partition_name:
                in_names.append(name)
        elif alloc.kind == "ExternalOutput":
            shape = tuple(alloc.tensor_shape)
            dtype = _mybir.dt.np(alloc.dtype)
            out_names.append(name)
            out_avals.append(jax.core.ShapedArray(shape, dtype))
            zero_outs.append(np.zeros(shape, dtype))
    n_params = len(in_names)
    n_outs = len(out_avals)
    in_names = in_names + out_names
    if partition_name is not None:
        in_names.append(partition_name)
    donate = tuple(range(n_params, n_params + n_outs))

    def _body(*args):
        operands = list(args)
        if partition_name is not None:
            operands.append(partition_id_tensor())
        return tuple(_bass_exec_p.bind(
            *operands,
            out_avals=tuple(out_avals),
            in_names=tuple(in_names),
            out_names=tuple(out_names),
            lowering_input_output_aliases=(),
            sim_require_finite=True,
            sim_require_nnan=True,
            nc=nc,
        ))

    n_cores = len(dev_ids)
    devices = [jax.devices()[i] for i in dev_ids]
    mesh = Mesh(np.asarray(devices), ("core",))
    in_specs = (PartitionSpec("core"),) * (n_params + n_outs)
    out_specs = (PartitionSpec("core"),) * n_outs
    sharded = jax.jit(
        shard_map(_body, mesh=mesh, in_specs=in_specs, out_specs=out_specs,
                  check_rep=False),
        donate_argnums=donate, keep_unused=True,
    )
    per_core = [[np.asarray(m[name]) for name in in_names[:n_params]]
                for m in in_maps]
    concat_in = [np.concatenate([per_core[c][i] for c in range(n_cores)],
                                axis=0) for i in range(n_params)]
    concat_zeros = [np.zeros((n_cores * z.shape[0], *z.shape[1:]), z.dtype)
                    for z in zero_outs]
    out_arrs = sharded(*concat_in, *concat_zeros)
    return out_arrs, out_names, out_avals, n_cores


def _assemble(out_arrs, out_names, out_avals, n_cores):
    return [
        {name: np.asarray(out_arrs[i]).reshape(n_cores, *out_avals[i].shape)[c]
         for i, name in enumerate(out_names)}
        for c in range(n_cores)
    ]


def _core_slices():
    """Global sample ranges per core id, heavy/light interleaved."""
    sizes = [BS_L if c in LIGHT_CORES else BS_H for c in range(N_CORES)]
    starts = np.concatenate([[0], np.cumsum(sizes)])
    return {c: slice(int(starts[c]), int(starts[c + 1]))
            for c in range(N_CORES)}


class _TraceResult:
    def __init__(self):
        self.exec_time_ns = None
        self.mean_exec_time_ns = None
        self.instructions_and_trace = None
        self.per_core = {}


def _trace_postprocess(neff_dir, nc_h, nc_l):
    """Convert each executable's NTFFs with its own NEFF (filenames pair
    them) and aggregate per-core exec times. Device indices in NTFF names
    are mesh-local; executables are created in dispatch order (heavy
    first), so map local->global through HEAVY_CORES/LIGHT_CORES."""
    import glob as _glob
    import json as _json
    import os
    import re
    import subprocess

    res = _TraceResult()
    stems = sorted(
        os.path.basename(p)[:-len(".neff")]
        for p in _glob.glob(os.path.join(neff_dir, "*_body*.neff"))
    )
    core_maps = [HEAVY_CORES, LIGHT_CORES]  # dispatch order
    times = {}
    for stem, cores in zip(stems, core_maps):
        for f in _glob.glob(os.path.join(neff_dir, stem + "-device*.ntff")):
            local = int(re.search(r"device(\d+)", os.path.basename(f)).group(1))
            gcore = cores[local]
            out_json = os.path.join(neff_dir, f"conv_{stem[-6:]}_{local}.json")
            try:
                subprocess.check_call(
                    ["neuron-profile", "view", "--ignore-nc-buf-usage",
                     "-s", os.path.basename(f), "-n", stem + ".neff",
                     "--output-format=json", f"--output-file={out_json}"],
                    cwd=neff_dir,
                    stdout=subprocess.DEVNULL, stderr=subprocess.DEVNULL,
                )
                d = _json.load(open(out_json))
                t_ns = int(round(d["summary"][0]["total_time"] * 1e9))
                times[gcore] = (t_ns, out_json)
            except Exception as e:
                print(f"ntff convert failed for {f}: {e}")
    if times:
        res.per_core = {c: t[0] for c, t in sorted(times.items())}
        worst = max(times, key=lambda c: times[c][0])
        res.exec_time_ns = times[worst][0]
        res.mean_exec_time_ns = sum(t[0] for t in times.values()) / len(times)
        res.instructions_and_trace = (None, times[worst][1])
    return res


def _run(mask_pred, theta_min, theta_max, trace=False, trace_kwargs=None,
         trace_cores=None):
    mask_pred = np.asarray(mask_pred, dtype=np.float32)
    theta_min = np.asarray(theta_min)
    theta_max = np.asarray(theta_max)
    T, w = _target_and_weight(theta_min, theta_max)

    slices = _core_slices()

    def in_map(c):
        sl = slices[c]
        x_core = np.ascontiguousarray(mask_pred[sl, 0]).reshape(
            sl.stop - sl.start, 128, Q * TH)
        return {"x": x_core, "tw": np.stack([T[sl], w[sl]])}

    nc_h = _get_nc(BS_H)
    nc_l = _get_nc(BS_L)
    maps_h = [in_map(c) for c in HEAVY_CORES]
    maps_l = [in_map(c) for c in LIGHT_CORES]

    # dispatch both programs (async); they run concurrently on
    # disjoint device sets
    trace_ctx = None
    if trace:
        import tempfile
        from antenv.axon_hooks import get_axon_ntff_profile_hook
        hook = get_axon_ntff_profile_hook()
        if hook is not None:
            neff_dir = tempfile.mkdtemp()
            trace_ctx = hook(neff_dir, list(range(N_CORES)))

    if trace_ctx is not None:
        with trace_ctx:
            arrs_h = _dispatch(nc_h, maps_h, HEAVY_CORES)
            arrs_l = _dispatch(nc_l, maps_l, LIGHT_CORES)
            res_h = _assemble(*arrs_h)
            res_l = _assemble(*arrs_l)
        trace_res = _trace_postprocess(neff_dir, nc_h, nc_l)
    else:
        arrs_h = _dispatch(nc_h, maps_h, HEAVY_CORES)
        arrs_l = _dispatch(nc_l, maps_l, LIGHT_CORES)
        res_h = _assemble(*arrs_h)
        res_l = _assemble(*arrs_l)
        trace_res = None

    per_sample = np.zeros(B, dtype=np.float64)
    for i, c in enumerate(HEAVY_CORES):
        per_sample[slices[c]] = res_h[i]["out"][:, 0]
    for i, c in enumerate(LIGHT_CORES):
        per_sample[slices[c]] = res_l[i]["out"][:, 0]
    total = per_sample.sum() / (TH * B)
    return np.float32(LAMBDA_ANG * total), trace_res


def kernel(mask_pred: np.ndarray, theta_min: np.ndarray,
           theta_max: np.ndarray) -> np.ndarray:
    loss, _ = _run(mask_pred, theta_min, theta_max)
    return np.asarray(loss, dtype=np.float32)


# revision 16
# speedup vs baseline: 1.1289x; 1.0125x over previous
"""Trainium2 Bass kernel for CMELossAngularProfileMSE_V2.

Strategy (pure data parallel over batch, 8 NeuronCores):
  - Shard B=128 samples -> 16 per core.
  - Per core, per sample: DMA the full [128, 5760] tile with a single
    dma_start (r-major within partition: partition p holds r in
    [16p, 16p+16), free dim = 16*360 contiguous; 23KB/partition
    descriptors halve DGE descriptor load vs half-sample chunks).
  - Fold 16 q-slices with a single DVE tree of just 3 ops
    (2880+1440+360-wide adds, ~5.3us/sample) leaving THREE slices;
    the TensorE absorbs the remaining merges: 3 one-hot matmuls
    accumulate the slices into PSUM row b (~4.8us/sample, one
    accumulation group). Every engine sits under the ~6.3us/sample
    wire cadence, so the pipeline is robustly DMA-bound (the old
    all-DVE fold at ~6.4us/sample was the limiter and its 5% deficit
    made cores collapse into stall limit-cycles; a GPSIMD offload
    fails differently: concurrent GPSIMD slows DVE ~4x via SBUF
    contention).
  - Host precomputes T' = R*T and w' = w/R^2 (exact power-of-two
    scalings of the Gaussian target / distance weight derived from
    theta_min/theta_max), so the device epilogue is just
    sum_theta((S - T')^2 * w') per sample -> out [16, 1], on DVE.
  - Host: loss = sum(all per-sample sums) / (360 * 128).
"""
import numpy as np

import concourse.bacc as bacc
import concourse.tile as tile
from concourse import mybir
from concourse.bass_utils import run_bass_kernel_spmd

F32 = mybir.dt.float32
ADD = mybir.AluOpType.add

N_CORES = 8
B = 128            # full batch
BS = B // N_CORES  # samples per core (16)
R = 2048
TH = 360
Q = 16             # r-slices per partition (2048 = 128 * 16)
SIGMA = 10.0
ALPHA_WEIGHT = 2.0
LAMBDA_ANG = 1.0

H = (Q // 2) * TH  # half-sample width (2880)


def _build_nc():
    nc = bacc.Bacc("TRN2", target_bir_lowering=False, debug=False)
    x = nc.dram_tensor("x", [BS, 128, Q * TH], F32, kind="ExternalInput").ap()
    tw = nc.dram_tensor("tw", [2, BS, TH], F32, kind="ExternalInput").ap()
    out = nc.dram_tensor("out", [BS, 1], F32, kind="ExternalOutput").ap()

    from contextlib import ExitStack
    with tile.TileContext(nc) as tc, ExitStack() as ctx:
        consts = ctx.enter_context(tc.tile_pool(name="consts", bufs=1))
        inp = ctx.enter_context(tc.tile_pool(name="inp", bufs=6))
        tailp = ctx.enter_context(tc.tile_pool(name="tailp", bufs=1))
        psum = ctx.enter_context(tc.tile_pool(name="psum", bufs=1, space="PSUM"))
        small = ctx.enter_context(tc.tile_pool(name="small", bufs=1))

        # one-hot weight matrices: O[:, b, j] = 1 if j == b else 0
        # (PSUM out base partition must be 0/32/64, so per-row matmuls
        # need the one-hot trick; built on gpsimd, idle at startup)
        O = consts.tile([128, BS, BS], F32)
        nc.gpsimd.memset(O[:], 0.0)
        for b in range(BS):
            nc.gpsimd.memset(O[:, b, b:b + 1], 1.0)

        t16w16 = small.tile([BS, 2, TH], F32)
        t16 = t16w16[:, 0, :]
        w16 = t16w16[:, 1, :]

        ps = psum.tile([BS, TH], F32)

        def tree3(xt, b, start):
            """3-op DVE tree -> 2 slices -> 2 accumulating matmuls."""
            nc.vector.tensor_add(xt[:, 0:2880], xt[:, 0:2880],
                                 xt[:, 2880:5760])
            nc.vector.tensor_add(xt[:, 0:1440], xt[:, 0:1440],
                                 xt[:, 1440:2880])
            nc.vector.tensor_add(xt[:, 0:720], xt[:, 0:720],
                                 xt[:, 720:1440])
            nc.tensor.matmul(ps[:], O[:, b, :], xt[:, 0:360],
                             start=start, stop=False)
            nc.tensor.matmul(ps[:], O[:, b, :], xt[:, 360:720],
                             start=False, stop=False)

        # samples 0..13: one full-sample DMA each (the fold tree needs the
        # whole sample anyway; 23KB/partition descriptors halve DGE load)
        tiles = {}
        for b in range(BS - 2):
            xt = inp.tile([128, Q * TH], F32)
            nc.sync.dma_start(xt[:], x[b])
            if b == 0:
                # tw load dispatched early (second in queue) so it is
                # resident long before the epilogue needs it
                nc.sync.dma_start(t16w16[:],
                                  tw.rearrange("two b t -> b two t"))
            tree3(xt, b, start=(b == 0))

        # Last two samples: diminishing chunks with dispatch order ==
        # fold program order, so the in-order DVE queue drains folds as
        # chunks arrive and only ~1us of work trails the final byte.
        # (A single specialized last sample does not work: its "eager"
        # folds sit behind sample 14's folds, whose data arrives at the
        # very end, serializing an extra full fold+matmul into the tail.)
        xa = tailp.tile([128, Q * TH], F32)   # sample 14
        xb = tailp.tile([128, Q * TH], F32)   # sample 15
        ba, bb = BS - 2, BS - 1
        # interleaved chunk dispatch: H0(2880) Q2(1440) E6(720) E7(720)
        for xt, b in ((xa, ba), (xb, bb)):
            nc.sync.dma_start(xt[:, 0:H], x[b][:, 0:H])
        for xt, b in ((xa, ba), (xb, bb)):
            nc.sync.dma_start(xt[:, 2880:4320], x[b][:, 2880:4320])
        for xt, b in ((xa, ba), (xb, bb)):
            nc.sync.dma_start(xt[:, 4320:5040], x[b][:, 4320:5040])
        for xt, b in ((xa, ba), (xb, bb)):
            nc.sync.dma_start(xt[:, 5040:5760], x[b][:, 5040:5760])
        # H0 -> 2 slices -> 2 matmuls, as each H0 lands
        for xt, b in ((xa, ba), (xb, bb)):
            nc.vector.tensor_add(xt[:, 0:1440], xt[:, 0:1440],
                                 xt[:, 1440:2880])
            nc.vector.tensor_add(xt[:, 0:720], xt[:, 0:720],
                                 xt[:, 720:1440])
            nc.tensor.matmul(ps[:], O[:, b, :], xt[:, 0:360],
                             start=False, stop=False)
            nc.tensor.matmul(ps[:], O[:, b, :], xt[:, 360:720],
                             start=False, stop=False)
        # Q2 (slices 8-11) -> s1 at [2880:3240]
        for xt, b in ((xa, ba), (xb, bb)):
            nc.vector.tensor_add(xt[:, 2880:3600], xt[:, 2880:3600],
                                 xt[:, 3600:4320])
            nc.vector.tensor_add(xt[:, 2880:3240], xt[:, 2880:3240],
                                 xt[:, 3240:3600])
        # E6 (slices 12,13) folded into s1
        for xt, b in ((xa, ba), (xb, bb)):
            nc.vector.tensor_add(xt[:, 4320:4680], xt[:, 4320:4680],
                                 xt[:, 4680:5040])
            nc.vector.tensor_add(xt[:, 2880:3240], xt[:, 2880:3240],
                                 xt[:, 4320:4680])
        # E7 (slices 14,15): the last bytes on the wire
        for xt, b in ((xa, ba), (xb, bb)):
            nc.vector.tensor_add(xt[:, 5040:5400], xt[:, 5040:5400],
                                 xt[:, 5400:5760])
            nc.vector.tensor_add(xt[:, 2880:3240], xt[:, 2880:3240],
                                 xt[:, 5040:5400])
            nc.tensor.matmul(ps[:], O[:, b, :], xt[:, 2880:3240],
                             start=False, stop=(b == BS - 1))

        d16 = small.tile([BS, TH], F32)
        nc.vector.scalar_tensor_tensor(
            d16[:], ps[:], 1.0, t16,
            op0=mybir.AluOpType.mult, op1=mybir.AluOpType.subtract,
        )
        sq16 = small.tile([BS, TH], F32)
        nc.vector.scalar_tensor_tensor(
            sq16[:], d16[:], 1.0, d16[:],
            op0=mybir.AluOpType.mult, op1=mybir.AluOpType.mult,
        )
        sqw16 = small.tile([BS, TH], F32)
        red = small.tile([BS, 1], F32)
        nc.vector.scalar_tensor_tensor(
            sqw16[:], sq16[:], 1.0, w16,
            op0=mybir.AluOpType.mult, op1=mybir.AluOpType.mult,
            accum_out=red[:],
        )
        nc.sync.dma_start(out[:], red[:])
    nc.compile()
    return nc


def _target_and_weight(theta_min: np.ndarray, theta_max: np.ndarray):
    """Gaussian soft target T and distance weight w, [B, TH] float32 each.

    Mirrors the reference formulas (computed in float64, cast to float32;
    differences vs the f32 jax pipeline are O(1 ulp))."""
    theta = np.arange(TH, dtype=np.float64)[None, None, :]      # [1, 1, TH]
    tmin = theta_min.astype(np.float64)[:, :, None]             # [B, K, 1]
    tmax = theta_max.astype(np.float64)[:, :, None]

    center_wrap = np.mod(0.5 * (tmin + tmax + 360.0), 360.0)
    center_t = np.where(tmin <= tmax, 0.5 * (tmin + tmax), center_wrap)
    d = np.abs(theta - center_t)
    dist_t = np.minimum(d, 360.0 - d)                           # [B, K, TH]
    T = np.clip(np.exp(-0.5 * (dist_t / SIGMA) ** 2).sum(axis=1), 0.0, 1.0)

    center_w = (tmin + np.mod(tmax - tmin, 360.0)) / 2.0
    dw = np.abs(theta - center_w)
    dist_w = np.minimum(dw, 360.0 - dw)
    w = 1.0 + ALPHA_WEIGHT * (dist_w.max(axis=1) / 180.0)       # [B, TH]

    # Feed the device T' = R*T and w' = w/R^2 (both exact scalings by
    # powers of two) so it can use the raw radial sums S instead of the
    # mean A = S/R:  ((S - R*T)^2 * w/R^2) == ((A - T)^2 * w).
    Tp = (T * np.float32(R)).astype(np.float32)
    wp = (w / np.float32(R) ** 2).astype(np.float32)
    return Tp, wp


_NC_CACHE = None


def _get_nc():
    global _NC_CACHE
    if _NC_CACHE is None:
        _NC_CACHE = _build_nc()
    return _NC_CACHE


def _run(mask_pred, theta_min, theta_max, trace=False, trace_kwargs=None,
         trace_cores=None):
    mask_pred = np.asarray(mask_pred, dtype=np.float32)
    theta_min = np.asarray(theta_min)
    theta_max = np.asarray(theta_max)
    T, w = _target_and_weight(theta_min, theta_max)

    in_maps = []
    for i in range(N_CORES):
        sl = slice(i * BS, (i + 1) * BS)
        x_core = np.ascontiguousarray(mask_pred[sl, 0]).reshape(BS, 128, Q * TH)
        tw_core = np.stack([T[sl], w[sl]])
        in_maps.append({"x": x_core, "tw": tw_core})

    kwargs = {}
    if trace:
        kwargs["trace"] = True
        if trace_kwargs:
            kwargs["trace_kwargs"] = trace_kwargs
        if trace_cores is not None:
            kwargs["trace_cores"] = trace_cores
    res = run_bass_kernel_spmd(_get_nc(), in_maps, core_ids=list(range(N_CORES)),
                               **kwargs)
    per_sample = np.concatenate(
        [res.results[i]["out"][:, 0] for i in range(N_CORES)]
    )
    total = per_sample.astype(np.float64).sum() / (TH * B)
    return np.float32(LAMBDA_ANG * total), res


def kernel(mask_pred: np.ndarray, theta_min: np.ndarray,
           theta_max: np.ndarray) -> np.ndarray:
    loss, _ = _run(mask_pred, theta_min, theta_max)
    return np.asarray(loss, dtype=np.float32)
